# revision 16
# baseline (speedup 1.0000x reference)
"""ACDC channel-FFT module via two-level circulant splitting on 8 TRN2 cores.

Math: the reference is out = take(ifft(fft(x*A, ch) * D, ch) + bias, perm) / sqrt(C),
i.e. z = M xa with M = circ(ifft(D)) complex-circulant, xa = A*x.  A circulant
splits along FFT butterfly levels into half-size blocks:
    circ_1024(c) -> circ_512(S) (+) nega_512(N)    on (x+, x-) = (x0+x1, x0-x1)
    circ_512(S)  -> circ_256   (+) nega_256        on (x++, x+-)
applied separately to Re(c) and Im(c).  Per 512-col chunk this needs
2*(4+4+16) = 48 matmul passes instead of the dense formulation's 128, with the
butterflies / recombines as DVE tensor_tensor adds at the 2x fp16 rate.

Device per core (one batch element, data-parallel over batch): per chunk,
DMA x in -> DVE butterflies -> per side (re/im): matmuls into two 4-bank
[128,2048] PSUM tiles (group A = c256|n256, group B = nega512), ACT evicts
each group in a single big activation, DVE recombines level-2 then level-1,
plane DMAs out.  PSUM ping-pongs via a bufs=2 pool so the PE never waits.
Dummy warm-up matmuls run during the initial x DMA to hold the PE HAM clock
gate open.

A / perm / bias / (1/sqrt(C) * 1/FSCALE) fold into host prep exactly like the
dense baseline folded A into W: the device computes the full circulant
transform; the host cast applies the diagonal A, and assembly applies the
permutation gather, bias constant, and descale.
"""

import numpy as np

import concourse.bass as bass
import concourse.mybir as mybir
from concourse import bacc
from concourse.tile import TileContext
from concourse.bass_utils import run_bass_kernel_spmd

B, C, S = 8, 1024, 4096
P = 128
NCHUNK = 512
NCH = S // NCHUNK     # 8 chunks
FSCALE = 256.0
N_CORES = 8

_CACHE = {}


def _build_nc():
    nc = bacc.Bacc()
    # x host-swizzled + A-folded: x[p, sq, kt*512+s'] = A[ch]*x_b[ch, sq*512+s']
    x = nc.dram_tensor("x", [P, NCH, 8 * NCHUNK], mybir.dt.float16, kind="ExternalInput")
    # nega-512 blocks (re, im), lhsT: w512[m, kt, p, i] = N_m[i, kt*128+p]*FSCALE
    w512 = nc.dram_tensor("w512", [2, 4, P, 512], mybir.dt.float16, kind="ExternalInput")
    # 256 blocks (c256_re, n256_re, c256_im, n256_im), lhsT layout
    w256 = nc.dram_tensor("w256", [4, 2, P, 256], mybir.dt.float16, kind="ExternalInput")
    # out[sq, pl, p, t*512 + s'] = z_pl[t*128+p, sq*512+s']*FSCALE (fp16)
    out = nc.dram_tensor("out", [NCH, 2, P, 8 * NCHUNK], mybir.dt.float16, kind="ExternalOutput")

    with TileContext(nc) as tc:
        with (
            tc.tile_pool(name="persist", bufs=1) as pp,
            tc.tile_pool(name="xin", bufs=3) as xp,
            tc.tile_pool(name="mid", bufs=2) as mp,
            tc.tile_pool(name="oev", bufs=2) as op,
            tc.tile_pool(name="zout", bufs=2) as zp,
            tc.tile_pool(name="ps", bufs=2, space="PSUM") as ps,
        ):
            # PE warmup: dummy matmuls fill the HAM activity window while the
            # first x chunk streams in, so real matmuls start at 2.4 GHz.
            wu = pp.tile([P, P], mybir.dt.float16, tag="wu")
            nc.vector.memset(wu, 0.0)
            wups = ps.tile([P, 4 * NCHUNK], mybir.dt.float32, tag="pt")
            for _ in range(110):
                nc.tensor.matmul(wups[:, 0:P], lhsT=wu, rhs=wu, start=True, stop=True)

            xt = [None] * NCH

            def _load_x(sq):
                t = xp.tile([P, 8 * NCHUNK], mybir.dt.float16, tag=f"x{sq % 3}")
                nc.sync.dma_start(out=t, in_=x[:, sq, :])
                xt[sq] = t

            # x chunk 0 first so compute starts ASAP; weights ride the scalar
            # queue (idle at start) so they don't delay the x stream.
            _load_x(0)
            wn = [[None] * 4 for _ in range(2)]      # nega512 re/im, 4 kt
            wq = [[None, None] for _ in range(4)]    # 256-blocks, 2 kt
            for m in range(2):
                for kt in range(4):
                    t = pp.tile([P, 512], mybir.dt.float16, tag=f"wn{m}_{kt}")
                    nc.scalar.dma_start(out=t, in_=w512[m, kt])
                    wn[m][kt] = t
            for m in range(4):
                for kt in range(2):
                    t = pp.tile([P, 256], mybir.dt.float16, tag=f"wq{m}_{kt}")
                    nc.scalar.dma_start(out=t, in_=w256[m, kt])
                    wq[m][kt] = t
            _load_x(1)

            def _bfly(sq):
                """butterflies for chunk sq: x+/x- then x++/x+- (DVE @2x)."""
                xc = xt[sq]
                xpm = mp.tile([P, 8 * NCHUNK], mybir.dt.float16, tag="xpm")
                nc.vector.tensor_add(
                    xpm[:, 0 : 4 * NCHUNK],
                    xc[:, 0 : 4 * NCHUNK],
                    xc[:, 4 * NCHUNK : 8 * NCHUNK],
                )
                nc.vector.tensor_sub(
                    xpm[:, 4 * NCHUNK : 8 * NCHUNK],
                    xc[:, 0 : 4 * NCHUNK],
                    xc[:, 4 * NCHUNK : 8 * NCHUNK],
                )
                xq = mp.tile([P, 4 * NCHUNK], mybir.dt.float16, tag="xq")
                nc.vector.tensor_add(
                    xq[:, 0 : 2 * NCHUNK],
                    xpm[:, 0 : 2 * NCHUNK],
                    xpm[:, 2 * NCHUNK : 4 * NCHUNK],
                )
                nc.vector.tensor_sub(
                    xq[:, 2 * NCHUNK : 4 * NCHUNK],
                    xpm[:, 0 : 2 * NCHUNK],
                    xpm[:, 2 * NCHUNK : 4 * NCHUNK],
                )
                return xpm, xq

            bf = [None] * NCH
            bf[0] = _bfly(0)
            for sq in range(NCH):
                if sq + 2 < NCH:
                    _load_x(sq + 2)
                xpm, xq = bf[sq]

                for side in range(2):
                    # group A: o++ = c256 @ x++ (slices 0,1), o+- = n256 @ x+-
                    pa = ps.tile([P, 4 * NCHUNK], mybir.dt.float32, tag="pt")
                    for half in range(2):          # 0: c256/x++, 1: n256/x+-
                        m = 2 * side + half
                        for ot in range(2):
                            for kt in range(2):
                                nc.tensor.matmul(
                                    pa[:, bass.ts(2 * half + ot, NCHUNK)],
                                    lhsT=wq[m][kt][:, bass.ts(ot, P)],
                                    rhs=xq[:, bass.ts(2 * half + kt, NCHUNK)],
                                    start=(kt == 0),
                                    stop=(kt == 1),
                                )
                    evA = op.tile([P, 4 * NCHUNK], mybir.dt.float16, tag=f"eA{side}")
                    nc.scalar.activation(evA, pa, mybir.ActivationFunctionType.Identity)

                    # group B: o- = nega512 @ x-
                    pb = ps.tile([P, 4 * NCHUNK], mybir.dt.float32, tag="pt")
                    for ot in range(4):
                        for kt in range(4):
                            nc.tensor.matmul(
                                pb[:, bass.ts(ot, NCHUNK)],
                                lhsT=wn[side][kt][:, bass.ts(ot, P)],
                                rhs=xpm[:, bass.ts(4 + kt, NCHUNK)],
                                start=(kt == 0),
                                stop=(kt == 3),
                            )
                    evB = op.tile([P, 4 * NCHUNK], mybir.dt.float16, tag=f"eB{side}")
                    nc.scalar.activation(evB, pb, mybir.ActivationFunctionType.Identity)

                    # queue next chunk's butterflies ahead of this chunk's
                    # recombines so the PE never waits on DVE at chunk start
                    if side == 0 and sq + 1 < NCH and bf[sq + 1] is None:
                        bf[sq + 1] = _bfly(sq + 1)

                    # level-2 recombine on GPSIMD: o+ = [o++ + o+-, o++ - o+-]
                    opl = mp.tile([P, 4 * NCHUNK], mybir.dt.float16, tag=f"op{side}")
                    nc.gpsimd.tensor_add(
                        opl[:, 0 : 2 * NCHUNK],
                        evA[:, 0 : 2 * NCHUNK],
                        evA[:, 2 * NCHUNK : 4 * NCHUNK],
                    )
                    nc.gpsimd.tensor_sub(
                        opl[:, 2 * NCHUNK : 4 * NCHUNK],
                        evA[:, 0 : 2 * NCHUNK],
                        evA[:, 2 * NCHUNK : 4 * NCHUNK],
                    )
                    # level-1 recombine: z = [o+ + o-, o+ - o-]
                    zt = zp.tile([P, 8 * NCHUNK], mybir.dt.float16, tag=f"zt{side}")
                    nc.vector.tensor_add(zt[:, 0 : 4 * NCHUNK], opl, evB)
                    nc.vector.tensor_sub(zt[:, 4 * NCHUNK : 8 * NCHUNK], opl, evB)
                    nc.sync.dma_start(out=out[sq, side], in_=zt)
    nc.compile()
    return nc


def _get_nc():
    if "nc" not in _CACHE:
        _CACHE["nc"] = _build_nc()
    return _CACHE["nc"]


def _split_blocks(ker):
    """real kernel (len n) -> (circ_{n/2}, nega_{n/2}) dense float64."""
    h = len(ker) // 2
    kp = ker[:h] + ker[h:]
    km = ker[:h] - ker[h:]
    ii = np.arange(h)[:, None]
    jj = np.arange(h)[None, :]
    d = (ii - jj) % h
    Smat = 0.5 * kp[d]
    Nmat = 0.5 * np.where(ii >= jj, km[d], -km[d])
    return Smat, Nmat, 0.5 * kp


def _host_prep(x, A, D, bias, perm):
    x = np.asarray(x, dtype=np.float32)
    A64 = np.asarray(A, dtype=np.float64)
    D64 = np.asarray(D, dtype=np.float64)

    c = np.fft.ifft(D64)  # circulant kernel of F^-1 diag(D) F
    scale = FSCALE / np.sqrt(C)
    n512, b256 = [], []
    for g in (c.real, c.imag):
        _, N1, kp1 = _split_blocks(g)          # level 1: keep nega512
        C2, N2, _ = _split_blocks(kp1)         # level 2 on the circ-512 branch
        n512.append(N1 * scale)
        b256.extend([C2 * scale, N2 * scale])
    w512 = np.stack(
        [np.ascontiguousarray(m.T.reshape(4, P, 512)).astype(np.float16) for m in n512]
    )
    w256 = np.stack(
        [np.ascontiguousarray(m.T.reshape(2, P, 256)).astype(np.float16) for m in b256]
    )
    # A folded into the x cast (like the baseline folded A into W);
    # x[b, ch, s] -> [b, p, sq, kt*512+s']
    xa = x * A64.astype(np.float32)[None, :, None]
    x16 = np.ascontiguousarray(
        xa.astype(np.float16)
        .reshape(B, 8, P, NCH, NCHUNK)
        .transpose(0, 2, 3, 1, 4)
        .reshape(B, P, NCH, 8 * NCHUNK)
    )
    return x16, w512, w256


def _assemble(outs, bias, perm):
    """device planes -> complex64 full output with perm/bias/descale on host."""
    bias64 = np.asarray(bias, dtype=np.float64)
    perm = np.asarray(perm).astype(np.int64)
    # out[sq, pl, p, t*512 + s'] -> z[pl, ch=t*128+p, s=sq*512+s']
    full = np.stack(outs, axis=0).reshape(B, NCH, 2, P, 8, NCHUNK)
    z = full.transpose(0, 2, 4, 3, 1, 5).reshape(B, 2, C, S)
    zp = z[:, :, perm, :].astype(np.float32) * np.float32(1.0 / FSCALE)
    res = (zp[:, 0] + 1j * zp[:, 1]).astype(np.complex64)
    bterm = ((bias64[perm]) / np.sqrt(C)).astype(np.complex64)
    res += bterm[None, :, None]
    return res


def _run(x, A, D, bias, perm, trace=False):
    x16, w512, w256 = _host_prep(x, A, D, bias, perm)
    nc = _get_nc()
    in_maps = [{"x": x16[i], "w512": w512, "w256": w256} for i in range(N_CORES)]
    res = run_bass_kernel_spmd(nc, in_maps, core_ids=list(range(N_CORES)), trace=trace)
    outs = [np.asarray(res.results[i]["out"]) for i in range(N_CORES)]
    return _assemble(outs, bias, perm), res


def kernel(x, A, D, bias, perm):
    out, _ = _run(x, A, D, bias, perm, trace=False)
    return out


# revision 17
# speedup vs baseline: 1.4092x; 1.4092x over previous
"""ACDC channel-FFT module via two-level circulant splitting on 8 TRN2 cores.

Math: the reference is out = take(ifft(fft(x*A, ch) * D, ch) + bias, perm) / sqrt(C),
i.e. z = M xa with M = circ(ifft(D)) complex-circulant, xa = A*x.  A circulant
splits along FFT butterfly levels into half-size blocks:
    circ_1024(c) -> circ_512(S) (+) nega_512(N)    on (x+, x-) = (x0+x1, x0-x1)
    circ_512(S)  -> circ_256   (+) nega_256        on (x++, x+-)
applied separately to Re(c) and Im(c).  Per 512-col chunk this needs
2*(4+4+16) = 48 matmul passes instead of the dense formulation's 128, with the
butterflies / recombines as DVE tensor_tensor adds at the 2x fp16 rate.

Device per core (one batch element, data-parallel over batch): per chunk,
DMA x in -> DVE butterflies -> per side (re/im): matmuls into two 4-bank
[128,2048] PSUM tiles (group A = c256|n256, group B = nega512), ACT evicts
each group in a single big activation, DVE recombines level-2 then level-1,
plane DMAs out.  PSUM ping-pongs via a bufs=2 pool so the PE never waits.
Dummy warm-up matmuls run during the initial x DMA to hold the PE HAM clock
gate open.

A / perm / bias / (1/sqrt(C) * 1/FSCALE) fold into host prep exactly like the
dense baseline folded A into W: the device computes the full circulant
transform; the host cast applies the diagonal A, and assembly applies the
permutation gather, bias constant, and descale.
"""

import numpy as np

import concourse.bass as bass
import concourse.mybir as mybir
from concourse import bacc
from concourse.tile import TileContext
from concourse.bass_utils import run_bass_kernel_spmd

B, C, S = 8, 1024, 4096
P = 128
NCHUNK = 512
NCH = S // NCHUNK     # 8 chunks
FSCALE = 256.0
N_CORES = 8

_CACHE = {}


def _build_nc():
    nc = bacc.Bacc()
    # x host-swizzled + A-folded: x[p, sq, kt*512+s'] = A[ch]*x_b[ch, sq*512+s']
    x = nc.dram_tensor("x", [P, NCH, 8 * NCHUNK], mybir.dt.float16, kind="ExternalInput")
    # nega-512 blocks (re, im), lhsT: w512[m, kt, p, i] = N_m[i, kt*128+p]*FSCALE
    w512 = nc.dram_tensor("w512", [2, 4, P, 512], mybir.dt.float16, kind="ExternalInput")
    # 256 blocks (c256_re, n256_re, c256_im, n256_im), lhsT layout
    w256 = nc.dram_tensor("w256", [4, 2, P, 256], mybir.dt.float16, kind="ExternalInput")
    # out[sq, pl, p, t*512 + s'] = z_pl[t*128+p, sq*512+s']*FSCALE (fp16)
    out = nc.dram_tensor("out", [NCH, 2, P, 8 * NCHUNK], mybir.dt.float16, kind="ExternalOutput")

    with TileContext(nc) as tc:
        with (
            tc.tile_pool(name="persist", bufs=1) as pp,
            tc.tile_pool(name="xin", bufs=3) as xp,
            tc.tile_pool(name="mid", bufs=2) as mp,
            tc.tile_pool(name="oev", bufs=2) as op,
            tc.tile_pool(name="zout", bufs=2) as zp,
            tc.tile_pool(name="ps", bufs=2, space="PSUM") as ps,
        ):
            # PE warmup: dummy matmuls fill the HAM activity window while the
            # first x chunk streams in, so real matmuls start at 2.4 GHz.
            wu = pp.tile([P, P], mybir.dt.float16, tag="wu")
            nc.vector.memset(wu, 0.0)
            wups = ps.tile([P, 4 * NCHUNK], mybir.dt.float32, tag="pt")
            for _ in range(110):
                nc.tensor.matmul(wups[:, 0:P], lhsT=wu, rhs=wu, start=True, stop=True)

            xt = [None] * NCH

            def _load_x(sq):
                t = xp.tile([P, 8 * NCHUNK], mybir.dt.float16, tag=f"x{sq % 3}")
                nc.sync.dma_start(out=t, in_=x[:, sq, :])
                xt[sq] = t

            # x chunk 0 first so compute starts ASAP; weights ride the scalar
            # queue (idle at start) so they don't delay the x stream.
            _load_x(0)
            wn = [[None] * 4 for _ in range(2)]      # nega512 re/im, 4 kt
            wq = [[None, None] for _ in range(4)]    # 256-blocks, 2 kt
            for m in range(2):
                for kt in range(4):
                    t = pp.tile([P, 512], mybir.dt.float16, tag=f"wn{m}_{kt}")
                    nc.scalar.dma_start(out=t, in_=w512[m, kt])
                    wn[m][kt] = t
            for m in range(4):
                for kt in range(2):
                    t = pp.tile([P, 256], mybir.dt.float16, tag=f"wq{m}_{kt}")
                    nc.scalar.dma_start(out=t, in_=w256[m, kt])
                    wq[m][kt] = t
            _load_x(1)

            def _bfly(sq):
                """butterflies for chunk sq: x+/x- then x++/x+- (DVE @2x)."""
                xc = xt[sq]
                xpm = mp.tile([P, 8 * NCHUNK], mybir.dt.float16, tag="xpm")
                nc.vector.tensor_add(
                    xpm[:, 0 : 4 * NCHUNK],
                    xc[:, 0 : 4 * NCHUNK],
                    xc[:, 4 * NCHUNK : 8 * NCHUNK],
                )
                nc.vector.tensor_sub(
                    xpm[:, 4 * NCHUNK : 8 * NCHUNK],
                    xc[:, 0 : 4 * NCHUNK],
                    xc[:, 4 * NCHUNK : 8 * NCHUNK],
                )
                xq = mp.tile([P, 4 * NCHUNK], mybir.dt.float16, tag="xq")
                nc.vector.tensor_add(
                    xq[:, 0 : 2 * NCHUNK],
                    xpm[:, 0 : 2 * NCHUNK],
                    xpm[:, 2 * NCHUNK : 4 * NCHUNK],
                )
                nc.vector.tensor_sub(
                    xq[:, 2 * NCHUNK : 4 * NCHUNK],
                    xpm[:, 0 : 2 * NCHUNK],
                    xpm[:, 2 * NCHUNK : 4 * NCHUNK],
                )
                return xpm, xq

            bf = [None] * NCH
            bf[0] = _bfly(0)
            for sq in range(NCH):
                if sq + 2 < NCH:
                    _load_x(sq + 2)
                xpm, xq = bf[sq]

                for side in range(2):
                    # group A: o++ = c256 @ x++ (slices 0,1), o+- = n256 @ x+-
                    pa = ps.tile([P, 4 * NCHUNK], mybir.dt.float32, tag="pt")
                    for half in range(2):          # 0: c256/x++, 1: n256/x+-
                        m = 2 * side + half
                        for ot in range(2):
                            for kt in range(2):
                                nc.tensor.matmul(
                                    pa[:, bass.ts(2 * half + ot, NCHUNK)],
                                    lhsT=wq[m][kt][:, bass.ts(ot, P)],
                                    rhs=xq[:, bass.ts(2 * half + kt, NCHUNK)],
                                    start=(kt == 0),
                                    stop=(kt == 1),
                                )
                    evA = op.tile([P, 4 * NCHUNK], mybir.dt.float16, tag=f"eA{side}")
                    nc.scalar.activation(evA, pa, mybir.ActivationFunctionType.Identity)

                    # group B: o- = nega512 @ x-
                    pb = ps.tile([P, 4 * NCHUNK], mybir.dt.float32, tag="pt")
                    for ot in range(4):
                        for kt in range(4):
                            nc.tensor.matmul(
                                pb[:, bass.ts(ot, NCHUNK)],
                                lhsT=wn[side][kt][:, bass.ts(ot, P)],
                                rhs=xpm[:, bass.ts(4 + kt, NCHUNK)],
                                start=(kt == 0),
                                stop=(kt == 3),
                            )
                    evB = op.tile([P, 4 * NCHUNK], mybir.dt.float16, tag=f"eB{side}")
                    nc.scalar.activation(evB, pb, mybir.ActivationFunctionType.Identity)

                    # queue next chunk's butterflies ahead of this chunk's
                    # recombines so the PE never waits on DVE at chunk start
                    if side == 0 and sq + 1 < NCH and bf[sq + 1] is None:
                        bf[sq + 1] = _bfly(sq + 1)

                    # level-2 recombine: o+ = [o++ + o+-, o++ - o+-]
                    opl = mp.tile([P, 4 * NCHUNK], mybir.dt.float16, tag=f"op{side}")
                    nc.vector.tensor_add(
                        opl[:, 0 : 2 * NCHUNK],
                        evA[:, 0 : 2 * NCHUNK],
                        evA[:, 2 * NCHUNK : 4 * NCHUNK],
                    )
                    nc.vector.tensor_sub(
                        opl[:, 2 * NCHUNK : 4 * NCHUNK],
                        evA[:, 0 : 2 * NCHUNK],
                        evA[:, 2 * NCHUNK : 4 * NCHUNK],
                    )
                    # level-1 recombine: z = [o+ + o-, o+ - o-]
                    zt = zp.tile([P, 8 * NCHUNK], mybir.dt.float16, tag=f"zt{side}")
                    nc.vector.tensor_add(zt[:, 0 : 4 * NCHUNK], opl, evB)
                    nc.vector.tensor_sub(zt[:, 4 * NCHUNK : 8 * NCHUNK], opl, evB)
                    nc.sync.dma_start(out=out[sq, side], in_=zt)
    nc.compile()
    return nc


def _get_nc():
    if "nc" not in _CACHE:
        _CACHE["nc"] = _build_nc()
    return _CACHE["nc"]


def _split_blocks(ker):
    """real kernel (len n) -> (circ_{n/2}, nega_{n/2}) dense float64."""
    h = len(ker) // 2
    kp = ker[:h] + ker[h:]
    km = ker[:h] - ker[h:]
    ii = np.arange(h)[:, None]
    jj = np.arange(h)[None, :]
    d = (ii - jj) % h
    Smat = 0.5 * kp[d]
    Nmat = 0.5 * np.where(ii >= jj, km[d], -km[d])
    return Smat, Nmat, 0.5 * kp


def _host_prep(x, A, D, bias, perm):
    x = np.asarray(x, dtype=np.float32)
    A64 = np.asarray(A, dtype=np.float64)
    D64 = np.asarray(D, dtype=np.float64)

    c = np.fft.ifft(D64)  # circulant kernel of F^-1 diag(D) F
    scale = FSCALE / np.sqrt(C)
    n512, b256 = [], []
    for g in (c.real, c.imag):
        _, N1, kp1 = _split_blocks(g)          # level 1: keep nega512
        C2, N2, _ = _split_blocks(kp1)         # level 2 on the circ-512 branch
        n512.append(N1 * scale)
        b256.extend([C2 * scale, N2 * scale])
    w512 = np.stack(
        [np.ascontiguousarray(m.T.reshape(4, P, 512)).astype(np.float16) for m in n512]
    )
    w256 = np.stack(
        [np.ascontiguousarray(m.T.reshape(2, P, 256)).astype(np.float16) for m in b256]
    )
    # A folded into the x cast (like the baseline folded A into W);
    # x[b, ch, s] -> [b, p, sq, kt*512+s']
    xa = x * A64.astype(np.float32)[None, :, None]
    x16 = np.ascontiguousarray(
        xa.astype(np.float16)
        .reshape(B, 8, P, NCH, NCHUNK)
        .transpose(0, 2, 3, 1, 4)
        .reshape(B, P, NCH, 8 * NCHUNK)
    )
    return x16, w512, w256


def _assemble(outs, bias, perm):
    """device planes -> complex64 full output with perm/bias/descale on host."""
    bias64 = np.asarray(bias, dtype=np.float64)
    perm = np.asarray(perm).astype(np.int64)
    # out[sq, pl, p, t*512 + s'] -> z[pl, ch=t*128+p, s=sq*512+s']
    full = np.stack(outs, axis=0).reshape(B, NCH, 2, P, 8, NCHUNK)
    z = full.transpose(0, 2, 4, 3, 1, 5).reshape(B, 2, C, S)
    zp = z[:, :, perm, :].astype(np.float32) * np.float32(1.0 / FSCALE)
    res = (zp[:, 0] + 1j * zp[:, 1]).astype(np.complex64)
    bterm = ((bias64[perm]) / np.sqrt(C)).astype(np.complex64)
    res += bterm[None, :, None]
    return res


def _run(x, A, D, bias, perm, trace=False):
    x16, w512, w256 = _host_prep(x, A, D, bias, perm)
    nc = _get_nc()
    in_maps = [{"x": x16[i], "w512": w512, "w256": w256} for i in range(N_CORES)]
    res = run_bass_kernel_spmd(nc, in_maps, core_ids=list(range(N_CORES)), trace=trace)
    outs = [np.asarray(res.results[i]["out"]) for i in range(N_CORES)]
    return _assemble(outs, bias, perm), res


def kernel(x, A, D, bias, perm):
    out, _ = _run(x, A, D, bias, perm, trace=False)
    return out


# revision 19
# speedup vs baseline: 1.4871x; 1.0553x over previous
"""ACDC channel-FFT module via two-level circulant splitting on 8 TRN2 cores.

Math: the reference is out = take(ifft(fft(x*A, ch) * D, ch) + bias, perm) / sqrt(C),
i.e. z = M xa with M = circ(ifft(D)) complex-circulant, xa = A*x.  A circulant
splits along FFT butterfly levels into half-size blocks:
    circ_1024(c) -> circ_512(S) (+) nega_512(N)    on (x+, x-) = (x0+x1, x0-x1)
    circ_512(S)  -> circ_256   (+) nega_256        on (x++, x+-)
applied separately to Re(c) and Im(c).  Per 512-col chunk this needs
2*(4+4+16) = 48 matmul passes instead of the dense formulation's 128, with the
butterflies / recombines as DVE tensor_tensor adds at the 2x fp16 rate.

Device per core (one batch element, data-parallel over batch): per chunk,
DMA x in -> DVE butterflies -> per side (re/im): matmuls into two 4-bank
[128,2048] PSUM tiles (group A = c256|n256, group B = nega512), ACT evicts
each group in a single big activation, DVE recombines level-2 then level-1,
plane DMAs out.  PSUM ping-pongs via a bufs=2 pool so the PE never waits.
Dummy warm-up matmuls run during the initial x DMA to hold the PE HAM clock
gate open.

A / perm / bias / (1/sqrt(C) * 1/FSCALE) fold into host prep exactly like the
dense baseline folded A into W: the device computes the full circulant
transform; the host cast applies the diagonal A, and assembly applies the
permutation gather, bias constant, and descale.
"""

import numpy as np

import concourse.bass as bass
import concourse.mybir as mybir
from concourse import bacc
from concourse.tile import TileContext
from concourse.bass_utils import run_bass_kernel_spmd

B, C, S = 8, 1024, 4096
P = 128
NCHUNK = 512
NCH = S // NCHUNK     # 8 chunks
FSCALE = 256.0
N_CORES = 8

_CACHE = {}


def _build_nc():
    nc = bacc.Bacc()
    # x host-swizzled + A-folded: x[p, sq, kt*512+s'] = A[ch]*x_b[ch, sq*512+s']
    x = nc.dram_tensor("x", [P, NCH, 8 * NCHUNK], mybir.dt.float16, kind="ExternalInput")
    # nega-512 blocks (re, im), lhsT: w512[m, kt, p, i] = N_m[i, kt*128+p]*FSCALE
    w512 = nc.dram_tensor("w512", [2, 4, P, 512], mybir.dt.float16, kind="ExternalInput")
    # 256 blocks (c256_re, n256_re, c256_im, n256_im), lhsT layout
    w256 = nc.dram_tensor("w256", [4, 2, P, 256], mybir.dt.float16, kind="ExternalInput")
    # out[sq, pl, p, t*512 + s'] = z_pl[t*128+p, sq*512+s']*FSCALE (fp16)
    out = nc.dram_tensor("out", [NCH, 2, P, 8 * NCHUNK], mybir.dt.float16, kind="ExternalOutput")

    with TileContext(nc) as tc:
        with (
            tc.tile_pool(name="persist", bufs=1) as pp,
            tc.tile_pool(name="xin", bufs=3) as xp,
            tc.tile_pool(name="mid", bufs=2) as mp,
            tc.tile_pool(name="oev", bufs=2) as op,
            tc.tile_pool(name="zout", bufs=2) as zp,
            tc.tile_pool(name="ps", bufs=2, space="PSUM") as ps,
        ):
            # PE warmup: dummy matmuls fill the HAM activity window while the
            # first x chunk streams in, so real matmuls start at 2.4 GHz.
            wu = pp.tile([P, P], mybir.dt.float16, tag="wu")
            nc.vector.memset(wu, 0.0)
            wups = ps.tile([P, 4 * NCHUNK], mybir.dt.float32, tag="pt")
            for _ in range(145):
                nc.tensor.matmul(wups[:, 0:P], lhsT=wu, rhs=wu, start=True, stop=True)

            xt = [None] * NCH

            def _load_x(sq):
                t = xp.tile([P, 8 * NCHUNK], mybir.dt.float16, tag=f"x{sq % 3}")
                nc.sync.dma_start(out=t, in_=x[:, sq, :])
                xt[sq] = t

            # x chunk 0 first so compute starts ASAP; weights ride the scalar
            # queue (idle at start) so they don't delay the x stream.
            _load_x(0)
            wn = [[None] * 4 for _ in range(2)]      # nega512 re/im, 4 kt
            wq = [[None, None] for _ in range(4)]    # 256-blocks, 2 kt
            for m in range(2):
                for kt in range(4):
                    t = pp.tile([P, 512], mybir.dt.float16, tag=f"wn{m}_{kt}")
                    nc.scalar.dma_start(out=t, in_=w512[m, kt])
                    wn[m][kt] = t
            for m in range(4):
                for kt in range(2):
                    t = pp.tile([P, 256], mybir.dt.float16, tag=f"wq{m}_{kt}")
                    nc.scalar.dma_start(out=t, in_=w256[m, kt])
                    wq[m][kt] = t
            _load_x(1)

            def _bfly(sq):
                """butterflies for chunk sq: x+/x- then x++/x+- (DVE @2x)."""
                xc = xt[sq]
                xpm = mp.tile([P, 8 * NCHUNK], mybir.dt.float16, tag="xpm")
                nc.vector.tensor_add(
                    xpm[:, 0 : 4 * NCHUNK],
                    xc[:, 0 : 4 * NCHUNK],
                    xc[:, 4 * NCHUNK : 8 * NCHUNK],
                )
                nc.vector.tensor_sub(
                    xpm[:, 4 * NCHUNK : 8 * NCHUNK],
                    xc[:, 0 : 4 * NCHUNK],
                    xc[:, 4 * NCHUNK : 8 * NCHUNK],
                )
                xq = mp.tile([P, 4 * NCHUNK], mybir.dt.float16, tag="xq")
                nc.vector.tensor_add(
                    xq[:, 0 : 2 * NCHUNK],
                    xpm[:, 0 : 2 * NCHUNK],
                    xpm[:, 2 * NCHUNK : 4 * NCHUNK],
                )
                nc.vector.tensor_sub(
                    xq[:, 2 * NCHUNK : 4 * NCHUNK],
                    xpm[:, 0 : 2 * NCHUNK],
                    xpm[:, 2 * NCHUNK : 4 * NCHUNK],
                )
                return xpm, xq

            bf = [None] * NCH
            bf[0] = _bfly(0)
            for sq in range(NCH):
                if sq + 2 < NCH:
                    _load_x(sq + 2)
                xpm, xq = bf[sq]

                for side in range(2):
                    # group B first: o- = nega512 @ x- needs only the level-1
                    # butterfly, so the PE starts before xq is ready
                    pb = ps.tile([P, 4 * NCHUNK], mybir.dt.float32, tag="pt")
                    for ot in range(4):
                        for kt in range(4):
                            nc.tensor.matmul(
                                pb[:, bass.ts(ot, NCHUNK)],
                                lhsT=wn[side][kt][:, bass.ts(ot, P)],
                                rhs=xpm[:, bass.ts(4 + kt, NCHUNK)],
                                start=(kt == 0),
                                stop=(kt == 3),
                            )
                    evB = op.tile([P, 4 * NCHUNK], mybir.dt.float16, tag=f"eB{side}")
                    nc.scalar.activation(evB, pb, mybir.ActivationFunctionType.Identity)

                    # group A: o++ = c256 @ x++ (slices 0,1), o+- = n256 @ x+-
                    pa = ps.tile([P, 4 * NCHUNK], mybir.dt.float32, tag="pt")
                    for half in range(2):          # 0: c256/x++, 1: n256/x+-
                        m = 2 * side + half
                        for ot in range(2):
                            for kt in range(2):
                                nc.tensor.matmul(
                                    pa[:, bass.ts(2 * half + ot, NCHUNK)],
                                    lhsT=wq[m][kt][:, bass.ts(ot, P)],
                                    rhs=xq[:, bass.ts(2 * half + kt, NCHUNK)],
                                    start=(kt == 0),
                                    stop=(kt == 1),
                                )
                    evA = op.tile([P, 4 * NCHUNK], mybir.dt.float16, tag=f"eA{side}")
                    nc.scalar.activation(evA, pa, mybir.ActivationFunctionType.Identity)

                    # queue next chunk's butterflies ahead of this chunk's
                    # recombines so the PE never waits on DVE at chunk start
                    if side == 0 and sq + 1 < NCH and bf[sq + 1] is None:
                        bf[sq + 1] = _bfly(sq + 1)

                    # level-2 recombine: o+ = [o++ + o+-, o++ - o+-]
                    opl = mp.tile([P, 4 * NCHUNK], mybir.dt.float16, tag=f"op{side}")
                    nc.vector.tensor_add(
                        opl[:, 0 : 2 * NCHUNK],
                        evA[:, 0 : 2 * NCHUNK],
                        evA[:, 2 * NCHUNK : 4 * NCHUNK],
                    )
                    nc.vector.tensor_sub(
                        opl[:, 2 * NCHUNK : 4 * NCHUNK],
                        evA[:, 0 : 2 * NCHUNK],
                        evA[:, 2 * NCHUNK : 4 * NCHUNK],
                    )
                    # level-1 recombine: z = [o+ + o-, o+ - o-]
                    zt = zp.tile([P, 8 * NCHUNK], mybir.dt.float16, tag=f"zt{side}")
                    nc.vector.tensor_add(zt[:, 0 : 4 * NCHUNK], opl, evB)
                    nc.vector.tensor_sub(zt[:, 4 * NCHUNK : 8 * NCHUNK], opl, evB)
                    nc.sync.dma_start(out=out[sq, side], in_=zt)
    nc.compile()
    return nc


def _get_nc():
    if "nc" not in _CACHE:
        _CACHE["nc"] = _build_nc()
    return _CACHE["nc"]


def _split_blocks(ker):
    """real kernel (len n) -> (circ_{n/2}, nega_{n/2}) dense float64."""
    h = len(ker) // 2
    kp = ker[:h] + ker[h:]
    km = ker[:h] - ker[h:]
    ii = np.arange(h)[:, None]
    jj = np.arange(h)[None, :]
    d = (ii - jj) % h
    Smat = 0.5 * kp[d]
    Nmat = 0.5 * np.where(ii >= jj, km[d], -km[d])
    return Smat, Nmat, 0.5 * kp


def _host_prep(x, A, D, bias, perm):
    x = np.asarray(x, dtype=np.float32)
    A64 = np.asarray(A, dtype=np.float64)
    D64 = np.asarray(D, dtype=np.float64)

    c = np.fft.ifft(D64)  # circulant kernel of F^-1 diag(D) F
    scale = FSCALE / np.sqrt(C)
    n512, b256 = [], []
    for g in (c.real, c.imag):
        _, N1, kp1 = _split_blocks(g)          # level 1: keep nega512
        C2, N2, _ = _split_blocks(kp1)         # level 2 on the circ-512 branch
        n512.append(N1 * scale)
        b256.extend([C2 * scale, N2 * scale])
    w512 = np.stack(
        [np.ascontiguousarray(m.T.reshape(4, P, 512)).astype(np.float16) for m in n512]
    )
    w256 = np.stack(
        [np.ascontiguousarray(m.T.reshape(2, P, 256)).astype(np.float16) for m in b256]
    )
    # A folded into the x cast (like the baseline folded A into W);
    # x[b, ch, s] -> [b, p, sq, kt*512+s']
    xa = x * A64.astype(np.float32)[None, :, None]
    x16 = np.ascontiguousarray(
        xa.astype(np.float16)
        .reshape(B, 8, P, NCH, NCHUNK)
        .transpose(0, 2, 3, 1, 4)
        .reshape(B, P, NCH, 8 * NCHUNK)
    )
    return x16, w512, w256


def _assemble(outs, bias, perm):
    """device planes -> complex64 full output with perm/bias/descale on host."""
    bias64 = np.asarray(bias, dtype=np.float64)
    perm = np.asarray(perm).astype(np.int64)
    # out[sq, pl, p, t*512 + s'] -> z[pl, ch=t*128+p, s=sq*512+s']
    full = np.stack(outs, axis=0).reshape(B, NCH, 2, P, 8, NCHUNK)
    z = full.transpose(0, 2, 4, 3, 1, 5).reshape(B, 2, C, S)
    zp = z[:, :, perm, :].astype(np.float32) * np.float32(1.0 / FSCALE)
    res = (zp[:, 0] + 1j * zp[:, 1]).astype(np.complex64)
    bterm = ((bias64[perm]) / np.sqrt(C)).astype(np.complex64)
    res += bterm[None, :, None]
    return res


def _run(x, A, D, bias, perm, trace=False):
    x16, w512, w256 = _host_prep(x, A, D, bias, perm)
    nc = _get_nc()
    in_maps = [{"x": x16[i], "w512": w512, "w256": w256} for i in range(N_CORES)]
    res = run_bass_kernel_spmd(nc, in_maps, core_ids=list(range(N_CORES)), trace=trace)
    outs = [np.asarray(res.results[i]["out"]) for i in range(N_CORES)]
    return _assemble(outs, bias, perm), res


def kernel(x, A, D, bias, perm):
    out, _ = _run(x, A, D, bias, perm, trace=False)
    return out
